# revision 14
# baseline (speedup 1.0000x reference)
"""AttentionLSTM Trainium2 kernel: 8-core tensor-parallel over the 4H gate dim.

Math per step t (reference):
    scores = (h @ A_flat) / 32         # per-sample: [N,L]
    w = softmax(scores)
    attn = A_flat @ w                  # [N,H]
    a = x_t@Wx + h@Wh + attn@Wattn + b # [N,4H]
    i,f,o,g = split(a); c = sig(f)*c + sig(i)*tanh(g); h = sig(o)*tanh(c)

Sharding: core k owns h-columns [128k,128k+128) and computes the 4 gate
strips for those columns (512 of 4096 gate cols). c stays sharded. Per
step one AllGather shares each core's transposed h-chunk + partial
scores. attn@Wattn is restructured as sum_l w_l * B_l with
B_l = A_flat[:,:,l] @ Wattn (built on device in a prologue); the
weighted sum runs on the PE as 16 PSUM-accumulating matmuls with
diag(w_l) stationary ("diag trick"). x@Wx is precomputed on device.
"""

import os
import sys

sys.path.insert(0, "/opt/trn_rl_repo")

import numpy as np

import concourse.bass as bass
import concourse.tile as tile
from concourse import bacc, mybir
from concourse.bass_utils import run_bass_kernel_spmd

N, T, D, H = 128, 64, 1024, 1024
L = 16
NC = 8
HCK = H // NC          # h-cols per core = 128
GC = 4 * HCK           # gate cols per core = 512
KC = 8                 # 128-row contraction chunks in D/H
P = 128

F32 = mybir.dt.float32
F32R = mybir.dt.float32r

_cache = {}


def _build(t_steps: int):
    nc = bacc.Bacc(
        "TRN2",
        target_bir_lowering=False,
        debug=False,
        enable_asserts=False,
        num_devices=NC,
    )

    # ---- kernel I/O (per-core feeds prepared on host) ----
    xT = nc.dram_tensor("xT", [D, T * P], F32R, kind="ExternalInput")
    wx = nc.dram_tensor("wx", [D, GC], F32R, kind="ExternalInput")
    wh = nc.dram_tensor("wh", [H, GC], F32R, kind="ExternalInput")
    wat = nc.dram_tensor("wat", [H, GC], F32R, kind="ExternalInput")
    bia = nc.dram_tensor("bia", [P, GC], F32R, kind="ExternalInput")
    asc = nc.dram_tensor("asc", [P, L * HCK], F32, kind="ExternalInput")  # [n,l,hc]/32
    at = nc.dram_tensor("at", [H, L * P], F32R, kind="ExternalInput")    # [h, l, n]
    eyeT = nc.dram_tensor("eyeT", [P, P], F32R, kind="ExternalInput")
    out = nc.dram_tensor("out", [P, T * HCK], F32, kind="ExternalOutput")

    # ---- internal DRAM ----
    xw_dram = nc.dram_tensor("xw_dram", [T * P, GC], F32)
    n_ag = t_steps  # one AllGather per step (h0's gather is step 0's)
    bin_d = nc.dram_tensor("bin_d", [n_ag, P, P + L], F32)
    bout_d = nc.dram_tensor("bout_d", [n_ag, NC * P, P + L], F32,
                            addr_space="Shared")

    rg = [list(range(NC))]

    with tile.TileContext(nc) as tc:
        # ================= static pools =================
        with tc.tile_pool(name="static", bufs=1) as sp, \
             tc.tile_pool(name="state", bufs=1) as statep:
            wh_sb = []
            for m in range(KC):
                t_ = sp.tile([P, GC], F32R, tag=f"wh{m}")
                nc.sync.dma_start(t_[:], wh[m * P:(m + 1) * P, :])
                wh_sb.append(t_)
            eye = sp.tile([P, P], F32R, tag="eye")
            nc.sync.dma_start(eye[:], eyeT[:, :])
            bias_sb = sp.tile([P, GC], F32R, tag="bias")
            nc.sync.dma_start(bias_sb[:], bia[:, :])
            asc_sb = sp.tile([P, L * HCK], F32, tag="asc")
            nc.sync.dma_start(asc_sb[:], asc[:, :])
            B_sb = [sp.tile([P, GC], F32R, tag=f"B{l}", name=f"B{l}")
                    for l in range(L)]

            c_st = statep.tile([P, HCK], F32, tag="c")

            # ============== prologue: B build ==============
            with tc.tile_pool(name="atp", bufs=1) as atp, \
                 tc.tile_pool(name="bps", bufs=4, space="PSUM") as bps:
                at_sb = []
                wat_sb = []
                for m in range(KC):
                    a_ = atp.tile([P, L * P], F32R, tag=f"at{m}")
                    nc.sync.dma_start(a_[:], at[m * P:(m + 1) * P, :])
                    at_sb.append(a_)
                    w_ = atp.tile([P, GC], F32R, tag=f"wat{m}")
                    nc.sync.dma_start(w_[:], wat[m * P:(m + 1) * P, :])
                    wat_sb.append(w_)
                for l in range(L):
                    bp = bps.tile([P, GC], F32, tag="bps")
                    for m in range(KC):
                        nc.tensor.matmul(
                            bp[:], at_sb[m][:, l * P:(l + 1) * P], wat_sb[m][:],
                            start=(m == 0), stop=(m == KC - 1),
                        )
                    nc.vector.tensor_copy(B_sb[l][:], bp[:])

            # ============== prologue: XW build ==============
            with tc.tile_pool(name="xtp", bufs=16) as xtp, \
                 tc.tile_pool(name="wxp", bufs=1) as wxp, \
                 tc.tile_pool(name="xwps", bufs=4, space="PSUM") as xwps:
                wx_sb = []
                for m in range(KC):
                    w_ = wxp.tile([P, GC], F32R, tag=f"wx{m}")
                    nc.sync.dma_start(w_[:], wx[m * P:(m + 1) * P, :])
                    wx_sb.append(w_)
                for j in range(t_steps):
                    xp = xwps.tile([P, GC], F32, tag="xwps")
                    nc.tensor.matmul(xp[:], eye[:], bias_sb[:], start=True, stop=False)
                    for m in range(KC):
                        xt_ = xtp.tile([P, P], F32R, tag="xt")
                        nc.sync.dma_start(
                            xt_[:], xT[m * P:(m + 1) * P, j * P:(j + 1) * P])
                        nc.tensor.matmul(xp[:], xt_[:], wx_sb[m][:],
                                         start=False, stop=(m == KC - 1))
                    xs = xtp.tile([P, GC], F32, tag="xs", name="xs", bufs=3)
                    nc.vector.tensor_copy(xs[:], xp[:])
                    nc.sync.dma_start(xw_dram[j * P:(j + 1) * P, :], xs[:])

            # ============== h0/c0 init ==============
            # c0 = mean_l A[n,hc,l] = 2 * sum_l asc (asc pre-scaled by 1/32)
            with tc.tile_pool(name="initp", bufs=1) as initp:
                r_ = initp.tile([P, HCK], F32, tag="r")
                nc.vector.tensor_reduce(
                    r_[:],
                    asc_sb[:].rearrange("p (l c) -> p c l", l=L),
                    axis=mybir.AxisListType.X, op=mybir.AluOpType.add)
                nc.vector.tensor_scalar_mul(c_st[:], r_[:], 2.0)

            # ============== recurrent loop ==============
            with tc.tile_pool(name="hp", bufs=3) as hp, \
                 tc.tile_pool(name="htp", bufs=2) as htp, \
                 tc.tile_pool(name="dgp", bufs=2 * L) as dgp, \
                 tc.tile_pool(name="xwsb", bufs=3) as xwsb, \
                 tc.tile_pool(name="smp", bufs=3) as smp, \
                 tc.tile_pool(name="gp", bufs=2) as gp, \
                 tc.tile_pool(name="prodp", bufs=2) as prodp, \
                 tc.tile_pool(name="aps", bufs=2, space="PSUM") as apsp, \
                 tc.tile_pool(name="tps", bufs=2, space="PSUM") as tpsp:

                h_t = hp.tile([P, HCK], F32R, tag="h")
                nc.vector.tensor_copy(h_t[:], c_st[:])  # h0 = c0

                for t in range(t_steps):
                    # -- share: transpose h -> bounce; partial scores -> bounce
                    tp = tpsp.tile([P, P], F32R, tag="tp")
                    nc.tensor.transpose(tp[:], h_t[:], eye[:])
                    tps = htp.tile([P, P], F32, tag="tps", name="tps")
                    nc.scalar.copy(tps[:], tp[:])
                    nc.sync.dma_start(bin_d[t, :, 0:P], tps[:])

                    prod = prodp.tile([P, L * HCK], F32, tag="prod")
                    nc.vector.tensor_tensor(
                        prod[:],
                        h_t[:].unsqueeze(1).broadcast_to((P, L, HCK)),
                        asc_sb[:].rearrange("p (l c) -> p l c", l=L),
                        op=mybir.AluOpType.mult)
                    spart = smp.tile([P, L], F32, tag="spart")
                    nc.vector.tensor_reduce(
                        spart[:], prod[:].rearrange("p (l c) -> p l c", l=L),
                        axis=mybir.AxisListType.X, op=mybir.AluOpType.add)
                    nc.sync.dma_start(bin_d[t, :, P:P + L], spart[:])

                    nc.gpsimd.collective_compute(
                        "AllGather", mybir.AluOpType.bypass,
                        replica_groups=rg,
                        ins=[bin_d[t]], outs=[bout_d[t]])

                    # -- bring back gathered h^T chunks + partials
                    hT = []
                    for m in range(NC):
                        ht_ = htp.tile([P, P], F32R, tag=f"hT{m}")
                        nc.sync.dma_start(
                            ht_[:], bout_d[t, m * P:(m + 1) * P, 0:P].bitcast(F32R))
                        hT.append(ht_)
                    parts = smp.tile([P, NC * L], F32, tag="parts")
                    nc.sync.dma_start(
                        parts[:].rearrange("p (j l) -> p j l", j=NC),
                        bout_d[t].rearrange("(j n) c -> n j c", j=NC)[:, :, P:P + L])

                    # -- softmax over l
                    scr = smp.tile([P, L], F32, tag="scr")
                    nc.vector.tensor_reduce(
                        scr[:], parts[:].rearrange("p (j l) -> p l j", j=NC),
                        axis=mybir.AxisListType.X, op=mybir.AluOpType.add)
                    negm = smp.tile([P, 1], F32, tag="negm")
                    nc.vector.tensor_reduce(
                        negm[:], scr[:], axis=mybir.AxisListType.X,
                        op=mybir.AluOpType.max, negate=True)
                    ex = smp.tile([P, L], F32, tag="ex")
                    ssum = smp.tile([P, 1], F32, tag="ssum")
                    nc.scalar.activation(
                        ex[:], scr[:], mybir.ActivationFunctionType.Exp,
                        bias=negm[:], accum_out=ssum[:])
                    rcp = smp.tile([P, 1], F32, tag="rcp")
                    nc.vector.reciprocal(rcp[:], ssum[:])
                    wgt = smp.tile([P, L], F32, tag="wgt")
                    nc.vector.tensor_scalar_mul(wgt[:], ex[:], rcp[:])

                    # -- gates: a = XW_t + bias + h@Wh + sum_l w_l B_l
                    xw_t = xwsb.tile([P, GC], F32R, tag="xw")
                    nc.sync.dma_start(
                        xw_t[:], xw_dram[t * P:(t + 1) * P, :].bitcast(F32R))
                    ap_ = apsp.tile([P, GC], F32, tag="a")
                    nc.tensor.matmul(ap_[:], eye[:], xw_t[:], start=True, stop=False)
                    for m in range(NC):
                        nc.tensor.matmul(ap_[:], hT[m][:], wh_sb[m][:],
                                         start=False, stop=False)
                    for l in range(L):
                        dg = dgp.tile([P, P], F32R, tag="dg")
                        nc.vector.tensor_scalar_mul(dg[:], eye[:], wgt[:, l:l + 1])
                        nc.tensor.matmul(ap_[:], dg[:], B_sb[l][:],
                                         start=False, stop=(l == L - 1))

                    # -- activations + cell
                    sig = gp.tile([P, 3 * HCK], F32, tag="sig")
                    nc.scalar.activation(sig[:], ap_[:, 0:3 * HCK],
                                         mybir.ActivationFunctionType.Sigmoid)
                    tg = gp.tile([P, HCK], F32, tag="tg")
                    nc.scalar.activation(tg[:], ap_[:, 3 * HCK:GC],
                                         mybir.ActivationFunctionType.Tanh)
                    ig = gp.tile([P, HCK], F32, tag="ig")
                    nc.vector.tensor_mul(ig[:], sig[:, 0:HCK], tg[:])
                    fc = gp.tile([P, HCK], F32, tag="fc")
                    nc.vector.tensor_mul(fc[:], sig[:, HCK:2 * HCK], c_st[:])
                    nc.vector.tensor_add(c_st[:], fc[:], ig[:])
                    th = gp.tile([P, HCK], F32, tag="th")
                    nc.scalar.activation(th[:], c_st[:],
                                         mybir.ActivationFunctionType.Tanh)
                    h_t = hp.tile([P, HCK], F32R, tag="h")
                    nc.vector.tensor_mul(h_t[:], sig[:, 2 * HCK:3 * HCK], th[:])

                    nc.sync.dma_start(
                        out[:, t * HCK:(t + 1) * HCK].bitcast(F32R), h_t[:])

    nc.compile()
    return nc


def _prep_inputs(x, A, Wx, Wh, Wattn, b):
    x = np.asarray(x, np.float32)
    A = np.asarray(A, np.float32)
    Wx = np.asarray(Wx, np.float32)
    Wh = np.asarray(Wh, np.float32)
    Wattn = np.asarray(Wattn, np.float32)
    b = np.asarray(b, np.float32)
    A_flat = A.reshape(N, H, L)

    # x transposed: [d, t*128+n]
    xT = np.ascontiguousarray(x.transpose(2, 1, 0).reshape(D, T * N))
    # A^T for B build: [h, l*128+n]
    at = np.ascontiguousarray(A_flat.transpose(1, 2, 0).reshape(H, L * N))
    eye = np.eye(P, dtype=np.float32)

    in_maps = []
    for k in range(NC):
        cols = np.concatenate(
            [g * H + np.arange(k * HCK, (k + 1) * HCK) for g in range(4)])
        asc_k = np.ascontiguousarray(
            A_flat[:, k * HCK:(k + 1) * HCK, :].transpose(0, 2, 1)
            .reshape(N, L * HCK) / np.sqrt(np.float32(H)))
        in_maps.append({
            "xT": xT,
            "wx": np.ascontiguousarray(Wx[:, cols]),
            "wh": np.ascontiguousarray(Wh[:, cols]),
            "wat": np.ascontiguousarray(Wattn[:, cols]),
            "bia": np.ascontiguousarray(np.broadcast_to(b[cols], (P, GC))),
            "asc": asc_k,
            "at": at,
            "eyeT": eye,
        })
    return in_maps


def kernel(x, A, Wx, Wh, Wattn, b, t_steps=T):
    if t_steps not in _cache:
        _cache[t_steps] = _build(t_steps)
    nc = _cache[t_steps]
    in_maps = _prep_inputs(x, A, Wx, Wh, Wattn, b)
    res = run_bass_kernel_spmd(nc, in_maps, core_ids=list(range(NC)), trace=False)
    global LAST_EXEC_NS
    LAST_EXEC_NS = res.exec_time_ns
    outp = np.empty((N, t_steps, H), np.float32)
    for k in range(NC):
        o = res.results[k]["out"].reshape(N, T, HCK)
        outp[:, :, k * HCK:(k + 1) * HCK] = o[:, :t_steps, :]
    return outp


LAST_EXEC_NS = None
